# revision 27
# baseline (speedup 1.0000x reference)
"""HGNN (DGL-style hypergraph conv x3) Bass kernel for trn2, 8 NeuronCores.

Math (per layer, weights/bias W,b):
    out = (D_v^-1 B^T D_e^-1 B X) @ W + b         (+ relu / final log_softmax)
where B is the (edge x node) incidence matrix given by (node_idx, edge_idx)
pairs. W commutes past the row-wise-scaled aggregations.

Layout/sharding (v2):
  * nnz are partitioned by the NODE owner (1-D range partition of nodes,
    VPC=6250 per core). Both aggregation passes use the same nnz set:
      - edge pass: destinations = ALL 20000 edges (157 blocks of 128);
        sources = local node rows, gathered from the core's local bf16
        feature table. Result is a PARTIAL edge table (this core's nodes'
        contributions), scaled by 1/deg_e during the PSUM->SBUF copy.
      - one AllReduce(add) over the 8 cores turns partials into the full
        edge table (replicated per core).
      - node pass: destinations = local nodes (49 blocks); sources = edge
        rows gathered from the reduced edge table.
  * everything feature-ish is bf16 (tables, gathers, selection matrices,
    PE matmuls); PSUM accumulation is fp32; softmax path fp32.
  * W3 is applied BEFORE the layer-3 aggregations (it commutes), so layer 3
    moves 40-wide rows instead of 256-wide. The projection is fused into
    layer 2's output stage, which is computed TRANSPOSED ([feat, node]
    PSUM tiles) so no PE transposes are needed anywhere:
      layer1:  psum[feat_h, node] -> *rdv -> lhsT for W1 -> [node,256] ->
               +b1, relu -> vloc1 rows (bf16)
      layer2:  psum[feat_h, node] -> *rdv -> W2^T matmuls -> psum[Fo_h,node]
               -> relu+b2 (scalar act, per-partition bias) -> v2t ->
               W3 matmuls -> [node, 40] -> xw3 rows (bf16)
      layer3:  40-wide edge partial + AllReduce + node pass ->
               *rdv, +b3, log_softmax (fp32) -> out.

Segment sums run on the tensor engine: for each 128-nnz tile of the sorted
incidence stream, a 0/1 selection matrix S (built on the vector engine by
comparing per-nnz local segment ids against an iota row, in bf16) maps
gathered rows into a PSUM accumulator indexed by segment within a 128-wide
block. Padding slots carry segment id -1 and contribute nothing.
"""
import hashlib
import sys

import numpy as np

sys.path.insert(0, "/opt/trn_rl_repo")

V, E, NNZ = 50000, 20000, 500000
D = 256
F3 = 40
LP = 128                   # layer-3 row width padded to 256B (dma_gather)
NCORES = 8
VPC = V // NCORES          # 6250 nodes per core
P = 128
NBE = (E + P - 1) // P     # 157 edge blocks (full edge range, per core)
NBV = (VPC + P - 1) // P   # 49 local node blocks
TC = 8                     # 128-nnz tiles per gather chunk

EPAD = NBE * P             # 20096 rows in partial/reduced edge tables


def _wrap_idxs(idx2d):
    """[128, n_tiles] slot-layout indices -> dma_gather wrapped int16 layout
    [128, n_tiles*8]: per TC-tile chunk, flat k (=tile*128+p within chunk)
    lives at [k%16, ch*64*TC//8... ]; replicated down in 16-row groups."""
    n_tiles = idx2d.shape[1]
    flat = idx2d.T.reshape(-1)                      # k = t*128 + p
    cols = n_tiles * P // 16
    w16 = flat.reshape(cols, 16).T                  # [16, cols], k=(c*16+r)
    return np.tile(w16, (8, 1)).astype(np.int16)    # replicate for 8 Q7 cores


def _side_arrays(seg, src, n_blocks, TB):
    """Build [128, n_tiles] gather-index / local-segment-id arrays for one
    core's nnz stream sorted by destination segment `seg` (block = seg//128).
    TB[b] = padded tile count for block b (common across cores)."""
    n_tiles = sum(TB)
    idx = np.zeros((P, n_tiles), dtype=np.int32)
    luc = np.full((P, n_tiles), -1.0, dtype=np.float32)
    counts = np.bincount(seg // P, minlength=n_blocks)
    offs = np.concatenate([[0], np.cumsum(counts)])
    col = 0
    for b in range(n_blocks):
        lo, hi = offs[b], offs[b + 1]
        s = np.arange(hi - lo)
        t, p = s // P, s % P
        idx[p, col + t] = src[lo:hi]
        luc[p, col + t] = (seg[lo:hi] - P * b).astype(np.float32)
        col += TB[b]
    return idx, luc


def _preprocess(node_idx, edge_idx):
    import ml_dtypes
    bf16 = ml_dtypes.bfloat16

    ni = np.asarray(node_idx, dtype=np.int64)
    ei = np.asarray(edge_idx, dtype=np.int64)
    deg_e = np.bincount(ei, minlength=E)
    deg_v = np.bincount(ni, minlength=V)
    rde_full = (1.0 / np.maximum(deg_e, 1)).astype(np.float32)
    rdv_full = (1.0 / np.maximum(deg_v, 1)).astype(np.float32)

    # nnz owned by the core owning the node; two sort orders per core.
    e_sorted, v_sorted = [], []
    for c in range(NCORES):
        sel = (ni >= c * VPC) & (ni < (c + 1) * VPC)
        vl = ni[sel] - c * VPC            # local node id (source/edge pass,
        eg = ei[sel]                      #                dest/node pass)
        o = np.argsort(eg, kind="stable")
        e_sorted.append((eg[o], vl[o]))   # dest=global edge, src=local node
        o = np.argsort(vl, kind="stable")
        v_sorted.append((vl[o], eg[o]))   # dest=local node, src=global edge

    TBe = [0] * NBE
    for c in range(NCORES):
        cnt = np.bincount(e_sorted[c][0] // P, minlength=NBE)
        for b in range(NBE):
            TBe[b] = max(TBe[b], -(-int(cnt[b]) // P))
    TBe[-1] += (-sum(TBe)) % TC
    TE = sum(TBe)

    TBv = [0] * NBV
    for c in range(NCORES):
        cnt = np.bincount(v_sorted[c][0] // P, minlength=NBV)
        for b in range(NBV):
            TBv[b] = max(TBv[b], -(-int(cnt[b]) // P))
    TBv[-1] += (-sum(TBv)) % TC
    TV = sum(TBv)

    # 1/deg_e laid out [128, NBE] block-major (same for all cores)
    rde = np.ones((P, NBE), dtype=np.float32)
    rde.reshape(-1, order="F")[:E] = 0  # noop, keep shape clear
    for b in range(NBE):
        n = min(P, E - P * b)
        rde[:n, b] = rde_full[P * b: P * b + n]
        rde[n:, b] = 1.0

    per_core = []
    for c in range(NCORES):
        idxe, luce = _side_arrays(e_sorted[c][0], e_sorted[c][1], NBE, TBe)
        idxv, lucv = _side_arrays(v_sorted[c][0], v_sorted[c][1], NBV, TBv)
        idxe, idxv = _wrap_idxs(idxe), _wrap_idxs(idxv)
        rdv = np.ones((P, NBV), dtype=np.float32)
        rdvT = np.ones((P, NBV * P), dtype=np.float32)
        for b in range(NBV):
            n = min(P, VPC - P * b)
            vals = rdv_full[c * VPC + P * b: c * VPC + P * b + n]
            rdv[:n, b] = vals
            rdvT[:, P * b: P * b + n] = vals[None, :]
        per_core.append(dict(
            idxe=idxe, luce=luce.astype(bf16),
            idxv=idxv, lucv=lucv.astype(bf16),
            rdv=rdv, rdvT=rdvT.astype(bf16)))
    return dict(TBe=TBe, TBv=TBv, TE=TE, TV=TV, rde=rde, per_core=per_core)


def _flatten_blocks(TB):
    """[(block, is_first, is_last)] per tile."""
    out = []
    for b, T in enumerate(TB):
        for t in range(T):
            out.append((b, t == 0, t == T - 1))
    return out


def _build(meta, debug=None):
    """debug: None = full kernel; 'p0' = dump layer-0 edge partials;
    'e0' = dump layer-0 reduced edge table; 'v0' = dump vloc1;
    'x3' = dump xw3 (all bf16, via the out tensor)."""
    import concourse.bacc as bacc
    import concourse.bass as bass
    import concourse.mybir as mybir
    import concourse.tile as tile

    from concourse.library_config import mlp

    f32 = mybir.dt.float32
    bf16 = mybir.dt.bfloat16
    i16 = mybir.dt.int16
    TE, TV = meta["TE"], meta["TV"]
    tiles_e = _flatten_blocks(meta["TBe"])
    tiles_v = _flatten_blocks(meta["TBv"])

    nc = bacc.Bacc("TRN2", target_bir_lowering=False, debug=False,
                   num_devices=NCORES)

    xloc_d = nc.dram_tensor("xloc", [VPC, D], bf16, kind="ExternalInput")
    idxe_d = nc.dram_tensor("idxe", [P, TE * 8], i16, kind="ExternalInput")
    luce_d = nc.dram_tensor("luce", [P, TE], bf16, kind="ExternalInput")
    idxv_d = nc.dram_tensor("idxv", [P, TV * 8], i16, kind="ExternalInput")
    lucv_d = nc.dram_tensor("lucv", [P, TV], bf16, kind="ExternalInput")
    rde_d = nc.dram_tensor("rde", [P, NBE], f32, kind="ExternalInput")
    rdv_d = nc.dram_tensor("rdv", [P, NBV], f32, kind="ExternalInput")
    rdvT_d = nc.dram_tensor("rdvt", [P, NBV * P], bf16, kind="ExternalInput")
    w1_d = nc.dram_tensor("w1", [D, D], bf16, kind="ExternalInput")
    w2_d = nc.dram_tensor("w2", [D, D], bf16, kind="ExternalInput")
    w3_d = nc.dram_tensor("w3", [D, F3], bf16, kind="ExternalInput")
    b1_d = nc.dram_tensor("b1x", [P, D], f32, kind="ExternalInput")
    b2_d = nc.dram_tensor("b2x", [P, 2], f32, kind="ExternalInput")
    b3_d = nc.dram_tensor("b3x", [P, F3], f32, kind="ExternalInput")
    iota_d = nc.dram_tensor("iota", [P, P], bf16, kind="ExternalInput")
    if debug == "g0":
        out_d = nc.dram_tensor("out", [P, TC * (D + P)], bf16,
                               kind="ExternalOutput")
    elif debug in ("p0", "e0"):
        out_d = nc.dram_tensor("out", [EPAD, D], bf16, kind="ExternalOutput")
    elif debug == "v0":
        out_d = nc.dram_tensor("out", [VPC, D], bf16, kind="ExternalOutput")
    elif debug == "x3":
        out_d = nc.dram_tensor("out", [VPC, F3], bf16, kind="ExternalOutput")
    else:
        out_d = nc.dram_tensor("out", [VPC, F3], f32, kind="ExternalOutput")

    vloc1_d = nc.dram_tensor("vloc1", [VPC, D], bf16)
    xw3_d = nc.dram_tensor("xw3", [VPC, LP], bf16)
    part_d = [nc.dram_tensor(f"part{i}", [EPAD, (D, D, LP)[i]], bf16)
              for i in range(3)]
    etab_d = [nc.dram_tensor(f"etab{i}", [EPAD, (D, D, LP)[i]], bf16)
              for i in range(3)]
    groups = [list(range(NCORES))]

    with tile.TileContext(nc) as tc:
        with (
            tc.tile_pool(name="const", bufs=1) as cpool,
            tc.tile_pool(name="g", bufs=6) as gpool,
            tc.tile_pool(name="st", bufs=4) as spool,
            tc.tile_pool(name="eo", bufs=4) as eopool,
            tc.tile_pool(name="ht", bufs=3) as htpool,
            tc.tile_pool(name="v2", bufs=4) as v2pool,
            tc.tile_pool(name="ob", bufs=4) as obpool,
            tc.tile_pool(name="sm", bufs=2) as smpool,
            tc.tile_pool(name="pse", bufs=3, space="PSUM") as pspool,
            tc.tile_pool(name="psn", bufs=3, space="PSUM") as npool,
            tc.tile_pool(name="psw", bufs=2, space="PSUM") as wpool,
        ):
            def load_const(dram, shape, tag, dtype=f32):
                t = cpool.tile(shape, dtype, tag=tag)
                nc.sync.dma_start(out=t[:], in_=dram[:])
                return t

            nc.gpsimd.load_library(mlp)
            idxe_sb = load_const(idxe_d, [P, TE * 8], "idxe", i16)
            luce_sb = load_const(luce_d, [P, TE], "luce", bf16)
            idxv_sb = load_const(idxv_d, [P, TV * 8], "idxv", i16)
            lucv_sb = load_const(lucv_d, [P, TV], "lucv", bf16)
            rde_sb = load_const(rde_d, [P, NBE], "rde")
            rdv_sb = load_const(rdv_d, [P, NBV], "rdv")
            rdvT_sb = load_const(rdvT_d, [P, NBV * P], "rdvt", bf16)
            iota_sb = load_const(iota_d, [P, P], "iota", bf16)
            b1_sb = load_const(b1_d, [P, D], "b1")
            b2_sb = load_const(b2_d, [P, 2], "b2")
            b3_sb = load_const(b3_d, [P, F3], "b3")
            w1_sb, w2_sb = [], []
            for wd, ws, tag in ((w1_d, w1_sb, "w1"), (w2_d, w2_sb, "w2")):
                for h in range(2):
                    t = cpool.tile([P, D], bf16, tag=f"{tag}{h}")
                    nc.sync.dma_start(out=t[:], in_=wd[h * P:(h + 1) * P, :])
                    ws.append(t)
            w3_sb = []
            for h in range(2):
                t = cpool.tile([P, F3], bf16, tag=f"w3{h}")
                nc.sync.dma_start(out=t[:], in_=w3_d[h * P:(h + 1) * P, :])
                w3_sb.append(t)

            CW = TC * P // 16  # wrapped-idx columns per chunk

            def gather_chunks(table, idx_sb, luc_sb, tiles, n_tiles, TW,
                              on_tile):
                """Common chunked gather + S build + segsum matmul driver.
                TW = table row width (gather elem_size); on_tile(j, g, st, b,
                first, last) issues the per-128-nnz-tile matmuls."""
                for ch in range(n_tiles // TC):
                    g = gpool.tile([P, TC * TW], bf16, tag="g")
                    nc.gpsimd.dma_gather(
                        out_ap=g[:].rearrange("p (t f) -> p t f", f=TW),
                        in_ap=table[:],
                        idxs_ap=idx_sb[:, ch * CW:(ch + 1) * CW],
                        num_idxs=TC * P, num_idxs_reg=TC * P,
                        elem_size=TW,
                    )
                    st = spool.tile([P, TC * P], bf16, tag="st")
                    nc.vector.tensor_tensor(
                        out=st[:].rearrange("p (t i) -> p t i", i=P),
                        in0=luc_sb[:, ch * TC:(ch + 1) * TC]
                            .unsqueeze(2).to_broadcast([P, TC, P]),
                        in1=iota_sb[:].unsqueeze(1).to_broadcast([P, TC, P]),
                        op=mybir.AluOpType.is_equal,
                    )
                    for j in range(TC):
                        b, first, last = tiles[ch * TC + j]
                        on_tile(j, g, st, b, first, last)

            # ---------------- edge pass: local partials over full E
            # TW = table/partial row width; F = value width (<= TW)
            def edge_pass(table, F, TW, part, dump=None):
                psums = {}

                def on_tile(j, g, st, b, first, last):
                    if first:
                        psums[b] = pspool.tile([P, F], f32, tag="pse",
                                               name=f"pse{b}")
                    nc.tensor.matmul(
                        out=psums[b][:],
                        lhsT=st[:, j * P:(j + 1) * P],
                        rhs=g[:, j * TW:j * TW + F],
                        start=first, stop=last,
                    )
                    if last:
                        ps = psums.pop(b)
                        esb = eopool.tile([P, TW], bf16, tag="eo")
                        if TW > F:
                            nc.vector.memset(esb[:, F:TW], 0.0)
                        nc.vector.tensor_scalar_mul(esb[:, 0:F], ps[:],
                                                    rde_sb[:, b:b + 1])
                        tgt = part if dump is None else dump
                        nc.sync.dma_start(out=tgt[P * b:P * (b + 1), :],
                                          in_=esb[:])

                gather_chunks(table, idxe_sb, luce_sb, tiles_e, TE, TW,
                              on_tile)

            # ---------------- node pass, layers 0/1 (transposed psum)
            def node_pass_t(etab, layer):
                psums = {}

                def on_tile(j, g, st, b, first, last):
                    if first:
                        psums[b] = npool.tile([P, D], f32, tag="psn",
                                              name=f"pn{b}")
                    for h in range(2):
                        nc.tensor.matmul(
                            out=psums[b][:, h * P:(h + 1) * P],
                            lhsT=g[:, j * D + h * P: j * D + (h + 1) * P],
                            rhs=st[:, j * P:(j + 1) * P],
                            start=first, stop=last,
                        )
                    if last:
                        finish(b, psums.pop(b))

                def finish(b, ps):
                    cnt = min(P, VPC - P * b)
                    hts = htpool.tile([P, D], bf16, tag="ht")
                    for h in range(2):
                        nc.vector.tensor_tensor(
                            out=hts[:, h * P:(h + 1) * P],
                            in0=ps[:, h * P:(h + 1) * P],
                            in1=rdvT_sb[:, P * b:P * (b + 1)],
                            op=mybir.AluOpType.mult)
                    if layer == 0:
                        po = wpool.tile([P, D], f32, tag="psw")
                        nc.tensor.matmul(out=po[:], lhsT=hts[:, 0:P],
                                         rhs=w1_sb[0][:], start=True,
                                         stop=False)
                        nc.tensor.matmul(out=po[:], lhsT=hts[:, P:D],
                                         rhs=w1_sb[1][:], start=False,
                                         stop=True)
                        tmp = obpool.tile([P, D], bf16, tag="ob")
                        nc.vector.tensor_tensor(out=tmp[:], in0=po[:],
                                                in1=b1_sb[:],
                                                op=mybir.AluOpType.add)
                        osb = obpool.tile([P, D], bf16, tag="ob")
                        nc.scalar.activation(
                            out=osb[:], in_=tmp[:],
                            func=mybir.ActivationFunctionType.Relu)
                        tgt = out_d if debug == "v0" else vloc1_d
                        nc.sync.dma_start(out=tgt[P * b:P * b + cnt, :],
                                          in_=osb[:cnt, :])
                    else:
                        v2t = v2pool.tile([P, D], bf16, tag="v2")
                        ptt = wpool.tile([P, D], f32, tag="psw")
                        for oh in range(2):
                            pt = ptt[:, oh * P:(oh + 1) * P]
                            nc.tensor.matmul(
                                out=pt,
                                lhsT=w2_sb[0][:, oh * P:(oh + 1) * P],
                                rhs=hts[:, 0:P], start=True, stop=False)
                            nc.tensor.matmul(
                                out=pt,
                                lhsT=w2_sb[1][:, oh * P:(oh + 1) * P],
                                rhs=hts[:, P:D], start=False, stop=True)
                            nc.scalar.activation(
                                out=v2t[:, oh * P:(oh + 1) * P], in_=pt,
                                func=mybir.ActivationFunctionType.Relu,
                                bias=b2_sb[:, oh:oh + 1])
                        pp = wpool.tile([P, F3], f32, tag="psw")
                        nc.tensor.matmul(out=pp[:], lhsT=v2t[:, 0:P],
                                         rhs=w3_sb[0][:], start=True,
                                         stop=False)
                        nc.tensor.matmul(out=pp[:], lhsT=v2t[:, P:D],
                                         rhs=w3_sb[1][:], start=False,
                                         stop=True)
                        xsb = obpool.tile([P, F3], bf16, tag="xw")
                        nc.vector.tensor_copy(xsb[:], pp[:])
                        if debug == "x3":
                            nc.sync.dma_start(out=out_d[P * b:P * b + cnt, :],
                                              in_=xsb[:cnt, :])
                        else:
                            nc.sync.dma_start(
                                out=xw3_d[P * b:P * b + cnt, 0:F3],
                                in_=xsb[:cnt, :])

                gather_chunks(etab, idxv_sb, lucv_sb, tiles_v, TV, D,
                              on_tile)

            # ---------------- node pass, layer 2 (final + log_softmax)
            def node_pass_final(etab):
                psums = {}

                def on_tile(j, g, st, b, first, last):
                    if first:
                        psums[b] = wpool.tile([P, F3], f32, tag="psw",
                                              name=f"pf{b}")
                    nc.tensor.matmul(
                        out=psums[b][:],
                        lhsT=st[:, j * P:(j + 1) * P],
                        rhs=g[:, j * LP:j * LP + F3],
                        start=first, stop=last,
                    )
                    if last:
                        finish(b, psums.pop(b))

                def finish(b, ps):
                    cnt = min(P, VPC - P * b)
                    osb = smpool.tile([P, F3], f32, tag="osb")
                    nc.vector.tensor_scalar_mul(osb[:], ps[:],
                                                rdv_sb[:, b:b + 1])
                    nc.vector.tensor_tensor(out=osb[:], in0=osb[:],
                                            in1=b3_sb[:],
                                            op=mybir.AluOpType.add)
                    negmax = smpool.tile([P, 1], f32, tag="negmax")
                    nc.vector.tensor_reduce(
                        out=negmax[:], in_=osb[:], axis=mybir.AxisListType.X,
                        op=mybir.AluOpType.max, negate=True)
                    expt = smpool.tile([P, F3], f32, tag="expt")
                    sumexp = smpool.tile([P, 1], f32, tag="sumexp")
                    nc.scalar.activation(
                        out=expt[:], in_=osb[:],
                        func=mybir.ActivationFunctionType.Exp,
                        bias=negmax[:, 0:1], accum_out=sumexp[:, 0:1])
                    logsum = smpool.tile([P, 1], f32, tag="logsum")
                    nc.scalar.activation(
                        out=logsum[:], in_=sumexp[:],
                        func=mybir.ActivationFunctionType.Ln)
                    shift = smpool.tile([P, 1], f32, tag="shift")
                    nc.vector.tensor_sub(out=shift[:], in0=negmax[:],
                                         in1=logsum[:])
                    res = smpool.tile([P, F3], f32, tag="res")
                    nc.vector.tensor_scalar_add(res[:], osb[:],
                                                shift[:, 0:1])
                    nc.sync.dma_start(out=out_d[P * b:P * b + cnt, :],
                                      in_=res[:cnt, :])

                gather_chunks(etab, idxv_sb, lucv_sb, tiles_v, TV, LP,
                              on_tile)

            if debug == "g0":
                g = gpool.tile([P, TC * D], bf16, tag="g")
                nc.gpsimd.dma_gather(
                    out_ap=g[:].rearrange("p (t f) -> p t f", f=D),
                    in_ap=xloc_d[:],
                    idxs_ap=idxe_sb[:, 0:CW],
                    num_idxs=TC * P, num_idxs_reg=TC * P,
                    elem_size=D,
                )
                st = spool.tile([P, TC * P], bf16, tag="st")
                nc.vector.tensor_tensor(
                    out=st[:].rearrange("p (t i) -> p t i", i=P),
                    in0=luce_sb[:, 0:TC]
                        .unsqueeze(2).to_broadcast([P, TC, P]),
                    in1=iota_sb[:].unsqueeze(1).to_broadcast([P, TC, P]),
                    op=mybir.AluOpType.is_equal,
                )
                nc.sync.dma_start(out=out_d[:, 0:TC * D], in_=g[:])
                nc.sync.dma_start(out=out_d[:, TC * D:], in_=st[:])
                layers = []
            else:
                layers = list(range(3))

            # ---------------- the three layers
            for layer in layers:
                table = (xloc_d, vloc1_d, xw3_d)[layer]
                F = D if layer < 2 else F3
                TW = D if layer < 2 else LP
                edge_pass(table, F, TW, part_d[layer],
                          dump=out_d if (debug == "p0" and layer == 0)
                          else None)
                if debug == "p0":
                    break
                nc.gpsimd.collective_compute(
                    "AllReduce", mybir.AluOpType.add, replica_groups=groups,
                    ins=[part_d[layer][:].opt()],
                    outs=[etab_d[layer][:].opt()],
                )
                if debug == "e0":
                    for b in range(NBE):
                        t = eopool.tile([P, D], bf16, tag="eo")
                        nc.sync.dma_start(
                            out=t[:], in_=etab_d[0][P * b:P * (b + 1), :])
                        nc.sync.dma_start(
                            out=out_d[P * b:P * (b + 1), :], in_=t[:])
                    break
                if layer < 2:
                    node_pass_t(etab_d[layer], layer)
                else:
                    node_pass_final(etab_d[layer])
                if debug == "v0" and layer == 0:
                    break
                if debug == "x3" and layer == 1:
                    break
    nc.finalize()
    return nc


def build_in_maps(meta, X, W1, b1, W2, b2, W3, b3):
    import ml_dtypes
    bf16 = ml_dtypes.bfloat16

    X = np.asarray(X, dtype=np.float32)
    iota = np.broadcast_to(np.arange(P, dtype=np.float32),
                           (P, P)).astype(bf16)
    ws = [np.ascontiguousarray(np.asarray(w, np.float32)).astype(bf16)
          for w in (W1, W2, W3)]
    b1x = np.broadcast_to(np.asarray(b1, np.float32), (P, D)).copy()
    b2a = np.asarray(b2, np.float32)
    b2x = np.stack([b2a[0:P], b2a[P:2 * P]], axis=1).copy()  # [128, 2]
    b3x = np.broadcast_to(np.asarray(b3, np.float32), (P, F3)).copy()

    in_maps = []
    for c in range(NCORES):
        pc = meta["per_core"][c]
        xloc = np.ascontiguousarray(X[c * VPC:(c + 1) * VPC]).astype(bf16)
        in_maps.append({
            "xloc": xloc, "idxe": pc["idxe"], "luce": pc["luce"],
            "idxv": pc["idxv"], "lucv": pc["lucv"],
            "rde": meta["rde"], "rdv": pc["rdv"], "rdvt": pc["rdvT"],
            "w1": ws[0], "w2": ws[1], "w3": ws[2],
            "b1x": b1x, "b2x": b2x, "b3x": b3x,
            "iota": iota,
        })
    return in_maps


_CACHE = {}


def kernel(X, node_idx, edge_idx, W1, b1, W2, b2, W3, b3):
    from concourse import bass_utils

    ni = np.asarray(node_idx, dtype=np.int32)
    ei = np.asarray(edge_idx, dtype=np.int32)

    key = hashlib.sha1(ni.tobytes() + ei.tobytes()).hexdigest()
    if key not in _CACHE:
        meta = _preprocess(ni, ei)
        nc = _build(meta)
        _CACHE[key] = (meta, nc)
    meta, nc = _CACHE[key]

    in_maps = build_in_maps(meta, X, W1, b1, W2, b2, W3, b3)
    res = bass_utils.run_bass_kernel_spmd(nc, in_maps, list(range(NCORES)))
    return np.concatenate([res.results[c]["out"] for c in range(NCORES)],
                          axis=0)


# revision 36
# speedup vs baseline: 2.8560x; 2.8560x over previous
"""HGNN (DGL-style hypergraph conv x3) Bass kernel for trn2, 8 NeuronCores.

Math (per layer, weights/bias W,b):
    out = (D_v^-1 B^T D_e^-1 B X) @ W + b         (+ relu / final log_softmax)
where B is the (edge x node) incidence matrix given by (node_idx, edge_idx)
pairs. W commutes past the row-wise-scaled aggregations.

Layout/sharding (v2):
  * nnz are partitioned by the NODE owner (1-D range partition of nodes,
    VPC=6250 per core). Both aggregation passes use the same nnz set:
      - edge pass: destinations = ALL 20000 edges (157 blocks of 128);
        sources = local node rows, gathered from the core's local bf16
        feature table. Result is a PARTIAL edge table (this core's nodes'
        contributions), scaled by 1/deg_e during the PSUM->SBUF copy.
      - one AllReduce(add) over the 8 cores turns partials into the full
        edge table (replicated per core).
      - node pass: destinations = local nodes (49 blocks); sources = edge
        rows gathered from the reduced edge table.
  * everything feature-ish is bf16 (tables, gathers, selection matrices,
    PE matmuls); PSUM accumulation is fp32; softmax path fp32.
  * W3 is applied BEFORE the layer-3 aggregations (it commutes), so layer 3
    moves 40-wide rows instead of 256-wide. The projection is fused into
    layer 2's output stage, which is computed TRANSPOSED ([feat, node]
    PSUM tiles) so no PE transposes are needed anywhere:
      layer1:  psum[feat_h, node] -> *rdv -> lhsT for W1 -> [node,256] ->
               +b1, relu -> vloc1 rows (bf16)
      layer2:  psum[feat_h, node] -> *rdv -> W2^T matmuls -> psum[Fo_h,node]
               -> relu+b2 (scalar act, per-partition bias) -> v2t ->
               W3 matmuls -> [node, 40] -> xw3 rows (bf16)
      layer3:  40-wide edge partial + AllReduce + node pass ->
               *rdv, +b3, log_softmax (fp32) -> out.

Segment sums run on the tensor engine: for each 128-nnz tile of the sorted
incidence stream, a 0/1 selection matrix S (built on the vector engine by
comparing per-nnz local segment ids against an iota row, in bf16) maps
gathered rows into a PSUM accumulator indexed by segment within a 128-wide
block. Padding slots carry segment id -1 and contribute nothing.
"""
import hashlib
import sys

import numpy as np

sys.path.insert(0, "/opt/trn_rl_repo")

V, E, NNZ = 50000, 20000, 500000
D = 256
F3 = 40
LP = 40                    # layer-3 table row width (bf16 elements)
NCORES = 8
VPC = V // NCORES          # 6250 nodes per core
P = 128
NBE = (E + P - 1) // P     # 157 edge blocks (full edge range, per core)
NBV = (VPC + P - 1) // P   # 49 local node blocks
TC = 16                    # 128-nnz tiles per gather chunk
PAD_IDX = 1 << 30          # gather index for padding slots (skipped via
                           # bounds_check; S column is 0 so any stale finite
                           # SBUF value contributes nothing)


def _side_arrays(seg, src, n_blocks, TB):
    """Build [128, n_tiles] gather-index / local-segment-id arrays for one
    core's nnz stream sorted by destination segment `seg` (block = seg//128).
    TB[b] = padded tile count for block b (common across cores)."""
    n_tiles = sum(TB)
    idx = np.zeros((P, n_tiles), dtype=np.int32)
    luc = np.full((P, n_tiles), -1.0, dtype=np.float32)
    counts = np.bincount(seg // P, minlength=n_blocks)
    offs = np.concatenate([[0], np.cumsum(counts)])
    col = 0
    for b in range(n_blocks):
        lo, hi = offs[b], offs[b + 1]
        s = np.arange(hi - lo)
        t, p = s // P, s % P
        idx[p, col + t] = src[lo:hi]
        luc[p, col + t] = (seg[lo:hi] - P * b).astype(np.float32)
        col += TB[b]
    return idx, luc


def _preprocess(node_idx, edge_idx):
    import ml_dtypes
    bf16 = ml_dtypes.bfloat16

    ni = np.asarray(node_idx, dtype=np.int64)
    ei = np.asarray(edge_idx, dtype=np.int64)
    deg_e = np.bincount(ei, minlength=E)
    deg_v = np.bincount(ni, minlength=V)
    rde_full = (1.0 / np.maximum(deg_e, 1)).astype(np.float32)
    rdv_full = (1.0 / np.maximum(deg_v, 1)).astype(np.float32)

    # nnz owned by the core owning the node; two sort orders per core.
    e_sorted, v_sorted = [], []
    for c in range(NCORES):
        sel = (ni >= c * VPC) & (ni < (c + 1) * VPC)
        vl = ni[sel] - c * VPC            # local node id (source/edge pass,
        eg = ei[sel]                      #                dest/node pass)
        o = np.argsort(eg, kind="stable")
        e_sorted.append((eg[o], vl[o]))   # dest=global edge, src=local node
        o = np.argsort(vl, kind="stable")
        v_sorted.append((vl[o], eg[o]))   # dest=local node, src=global edge

    TBe = [0] * NBE
    for c in range(NCORES):
        cnt = np.bincount(e_sorted[c][0] // P, minlength=NBE)
        for b in range(NBE):
            TBe[b] = max(TBe[b], -(-int(cnt[b]) // P))
    TBe[-1] += (-sum(TBe)) % TC
    TE = sum(TBe)

    TBv = [0] * NBV
    for c in range(NCORES):
        cnt = np.bincount(v_sorted[c][0] // P, minlength=NBV)
        for b in range(NBV):
            TBv[b] = max(TBv[b], -(-int(cnt[b]) // P))
    TBv[-1] += (-sum(TBv)) % TC
    TV = sum(TBv)

    # 1/deg_e laid out [128, NBE] block-major (same for all cores)
    rde = np.ones((P, NBE), dtype=np.float32)
    rde.reshape(-1, order="F")[:E] = 0  # noop, keep shape clear
    for b in range(NBE):
        n = min(P, E - P * b)
        rde[:n, b] = rde_full[P * b: P * b + n]
        rde[n:, b] = 1.0

    per_core = []
    for c in range(NCORES):
        idxe, luce = _side_arrays(e_sorted[c][0], e_sorted[c][1], NBE, TBe)
        idxv, lucv = _side_arrays(v_sorted[c][0], v_sorted[c][1], NBV, TBv)
        rdv = np.ones((P, NBV), dtype=np.float32)
        rdvT = np.ones((P, NBV * P), dtype=np.float32)
        for b in range(NBV):
            n = min(P, VPC - P * b)
            vals = rdv_full[c * VPC + P * b: c * VPC + P * b + n]
            rdv[:n, b] = vals
            rdvT[:, P * b: P * b + n] = vals[None, :]
        per_core.append(dict(
            idxe=idxe, luce=luce.astype(bf16),
            idxv=idxv, lucv=lucv.astype(bf16),
            rdv=rdv, rdvT=rdvT.astype(bf16)))
    return dict(TBe=TBe, TBv=TBv, TE=TE, TV=TV, rde=rde, per_core=per_core)


def _flatten_blocks(TB):
    """[(block, is_first, is_last)] per tile."""
    out = []
    for b, T in enumerate(TB):
        for t in range(T):
            out.append((b, t == 0, t == T - 1))
    return out


def _build(meta, debug=None):
    """debug: None = full kernel; 'p0' = dump layer-0 edge partials;
    'e0' = dump layer-0 reduced edge table; 'v0' = dump vloc1;
    'x3' = dump xw3 (all bf16, via the out tensor)."""
    import concourse.bacc as bacc
    import concourse.bass as bass
    import concourse.mybir as mybir
    import concourse.tile as tile

    f32 = mybir.dt.float32
    bf16 = mybir.dt.bfloat16
    i32 = mybir.dt.int32
    TE, TV = meta["TE"], meta["TV"]
    tiles_e = _flatten_blocks(meta["TBe"])
    tiles_v = _flatten_blocks(meta["TBv"])

    nc = bacc.Bacc("TRN2", target_bir_lowering=False, debug=False,
                   num_devices=NCORES)

    xloc_d = nc.dram_tensor("xloc", [VPC, D], bf16, kind="ExternalInput")
    idxe_d = nc.dram_tensor("idxe", [P, TE], i32, kind="ExternalInput")
    luce_d = nc.dram_tensor("luce", [P, TE], bf16, kind="ExternalInput")
    idxv_d = nc.dram_tensor("idxv", [P, TV], i32, kind="ExternalInput")
    lucv_d = nc.dram_tensor("lucv", [P, TV], bf16, kind="ExternalInput")
    rde_d = nc.dram_tensor("rde", [P, NBE], f32, kind="ExternalInput")
    rdv_d = nc.dram_tensor("rdv", [P, NBV], f32, kind="ExternalInput")
    rdvT_d = nc.dram_tensor("rdvt", [P, NBV * P], bf16, kind="ExternalInput")
    w1_d = nc.dram_tensor("w1", [D, D], bf16, kind="ExternalInput")
    w2_d = nc.dram_tensor("w2", [D, D], bf16, kind="ExternalInput")
    w3_d = nc.dram_tensor("w3", [D, F3], bf16, kind="ExternalInput")
    b1_d = nc.dram_tensor("b1x", [P, D], f32, kind="ExternalInput")
    b2_d = nc.dram_tensor("b2x", [P, 2], f32, kind="ExternalInput")
    b3_d = nc.dram_tensor("b3x", [P, F3], f32, kind="ExternalInput")
    iota_d = nc.dram_tensor("iota", [P, P], bf16, kind="ExternalInput")
    if debug == "g0":
        out_d = nc.dram_tensor("out", [P, TC * (D + P)], bf16,
                               kind="ExternalOutput")
    elif debug in ("p0", "e0"):
        out_d = nc.dram_tensor("out", [EPAD, D], bf16, kind="ExternalOutput")
    elif debug == "v0":
        out_d = nc.dram_tensor("out", [VPC, D], bf16, kind="ExternalOutput")
    elif debug == "x3":
        out_d = nc.dram_tensor("out", [VPC, F3], bf16, kind="ExternalOutput")
    else:
        out_d = nc.dram_tensor("out", [VPC, F3], f32, kind="ExternalOutput")

    vloc1_d = nc.dram_tensor("vloc1", [VPC, D], bf16)
    xw3_d = nc.dram_tensor("xw3", [VPC, LP], bf16)
    part_d = [nc.dram_tensor(f"part{i}", [EPAD, (D, D, LP)[i]], bf16)
              for i in range(3)]
    etab_d = [nc.dram_tensor(f"etab{i}", [EPAD, (D, D, LP)[i]], bf16)
              for i in range(3)]
    groups = [list(range(NCORES))]

    with tile.TileContext(nc) as tc:
        with (
            tc.tile_pool(name="const", bufs=1) as cpool,
            tc.tile_pool(name="g", bufs=6) as gpool,
            tc.tile_pool(name="st", bufs=4) as spool,
            tc.tile_pool(name="eo", bufs=4) as eopool,
            tc.tile_pool(name="ht", bufs=3) as htpool,
            tc.tile_pool(name="v2", bufs=4) as v2pool,
            tc.tile_pool(name="ob", bufs=4) as obpool,
            tc.tile_pool(name="sm", bufs=2) as smpool,
            tc.tile_pool(name="pse", bufs=3, space="PSUM") as pspool,
            tc.tile_pool(name="psn", bufs=3, space="PSUM") as npool,
            tc.tile_pool(name="psw", bufs=2, space="PSUM") as wpool,
        ):
            def load_const(dram, shape, tag, dtype=f32):
                t = cpool.tile(shape, dtype, tag=tag)
                nc.sync.dma_start(out=t[:], in_=dram[:])
                return t

            idxe_sb = load_const(idxe_d, [P, TE], "idxe", i32)
            luce_sb = load_const(luce_d, [P, TE], "luce", bf16)
            idxv_sb = load_const(idxv_d, [P, TV], "idxv", i32)
            lucv_sb = load_const(lucv_d, [P, TV], "lucv", bf16)
            rde_sb = load_const(rde_d, [P, NBE], "rde")
            rdv_sb = load_const(rdv_d, [P, NBV], "rdv")
            rdvT_sb = load_const(rdvT_d, [P, NBV * P], "rdvt", bf16)
            iota_sb = load_const(iota_d, [P, P], "iota", bf16)
            b1_sb = load_const(b1_d, [P, D], "b1")
            b2_sb = load_const(b2_d, [P, 2], "b2")
            b3_sb = load_const(b3_d, [P, F3], "b3")
            w1_sb, w2_sb = [], []
            for wd, ws, tag in ((w1_d, w1_sb, "w1"), (w2_d, w2_sb, "w2")):
                for h in range(2):
                    t = cpool.tile([P, D], bf16, tag=f"{tag}{h}")
                    nc.sync.dma_start(out=t[:], in_=wd[h * P:(h + 1) * P, :])
                    ws.append(t)
            w3_sb = []
            for h in range(2):
                t = cpool.tile([P, F3], bf16, tag=f"w3{h}")
                nc.sync.dma_start(out=t[:], in_=w3_d[h * P:(h + 1) * P, :])
                w3_sb.append(t)

            def gather_chunks(table, idx_sb, luc_sb, tiles, n_tiles, TW,
                              on_tile):
                """Common chunked gather + S build + segsum matmul driver.
                TW = table row width in bf16 elems. The gather itself runs
                on the (proven) f32 indirect path over a bitcast view — the
                bf16 indirect lowering mis-strides multi-tile gathers."""
                for ch in range(n_tiles // TC):
                    gf = gpool.tile([P, TC * TW // 2], f32, tag="g")
                    nc.gpsimd.indirect_dma_start(
                        out=gf[:], out_offset=None,
                        in_=table[:].bitcast(f32),
                        in_offset=bass.IndirectOffsetOnAxis(
                            ap=idx_sb[:, ch * TC:(ch + 1) * TC], axis=0),
                    )
                    g = gf[:].bitcast(bf16)
                    st = spool.tile([P, TC * P], bf16, tag="st")
                    nc.vector.tensor_tensor(
                        out=st[:].rearrange("p (t i) -> p t i", i=P),
                        in0=luc_sb[:, ch * TC:(ch + 1) * TC]
                            .unsqueeze(2).to_broadcast([P, TC, P]),
                        in1=iota_sb[:].unsqueeze(1).to_broadcast([P, TC, P]),
                        op=mybir.AluOpType.is_equal,
                    )
                    for j in range(TC):
                        b, first, last = tiles[ch * TC + j]
                        on_tile(j, g, st, b, first, last)

            # ---------------- edge pass: local partials over full E
            # TW = table/partial row width; F = value width (<= TW)
            def edge_pass(table, F, TW, part, dump=None):
                psums = {}

                def on_tile(j, g, st, b, first, last):
                    if first:
                        psums[b] = pspool.tile([P, F], f32, tag="pse",
                                               name=f"pse{b}")
                    nc.tensor.matmul(
                        out=psums[b][:],
                        lhsT=st[:, j * P:(j + 1) * P],
                        rhs=g[:, j * TW:j * TW + F],
                        start=first, stop=last,
                    )
                    if last:
                        ps = psums.pop(b)
                        esb = eopool.tile([P, TW], bf16, tag="eo")
                        if TW > F:
                            nc.vector.memset(esb[:, F:TW], 0.0)
                        nc.vector.tensor_scalar_mul(esb[:, 0:F], ps[:],
                                                    rde_sb[:, b:b + 1])
                        tgt = part if dump is None else dump
                        nc.sync.dma_start(out=tgt[P * b:P * (b + 1), :],
                                          in_=esb[:])

                gather_chunks(table, idxe_sb, luce_sb, tiles_e, TE, TW,
                              on_tile)

            # ---------------- node pass, layers 0/1 (transposed psum)
            def node_pass_t(etab, layer):
                psums = {}

                def on_tile(j, g, st, b, first, last):
                    if first:
                        psums[b] = npool.tile([P, D], f32, tag="psn",
                                              name=f"pn{b}")
                    for h in range(2):
                        nc.tensor.matmul(
                            out=psums[b][:, h * P:(h + 1) * P],
                            lhsT=g[:, j * D + h * P: j * D + (h + 1) * P],
                            rhs=st[:, j * P:(j + 1) * P],
                            start=first, stop=last,
                        )
                    if last:
                        finish(b, psums.pop(b))

                def finish(b, ps):
                    cnt = min(P, VPC - P * b)
                    hts = htpool.tile([P, D], bf16, tag="ht")
                    for h in range(2):
                        nc.vector.tensor_tensor(
                            out=hts[:, h * P:(h + 1) * P],
                            in0=ps[:, h * P:(h + 1) * P],
                            in1=rdvT_sb[:, P * b:P * (b + 1)],
                            op=mybir.AluOpType.mult)
                    if layer == 0:
                        po = wpool.tile([P, D], f32, tag="psw")
                        nc.tensor.matmul(out=po[:], lhsT=hts[:, 0:P],
                                         rhs=w1_sb[0][:], start=True,
                                         stop=False)
                        nc.tensor.matmul(out=po[:], lhsT=hts[:, P:D],
                                         rhs=w1_sb[1][:], start=False,
                                         stop=True)
                        tmp = obpool.tile([P, D], bf16, tag="ob")
                        nc.vector.tensor_tensor(out=tmp[:], in0=po[:],
                                                in1=b1_sb[:],
                                                op=mybir.AluOpType.add)
                        osb = obpool.tile([P, D], bf16, tag="ob")
                        nc.scalar.activation(
                            out=osb[:], in_=tmp[:],
                            func=mybir.ActivationFunctionType.Relu)
                        tgt = out_d if debug == "v0" else vloc1_d
                        nc.sync.dma_start(out=tgt[P * b:P * b + cnt, :],
                                          in_=osb[:cnt, :])
                    else:
                        v2t = v2pool.tile([P, D], bf16, tag="v2")
                        ptt = wpool.tile([P, D], f32, tag="psw")
                        for oh in range(2):
                            pt = ptt[:, oh * P:(oh + 1) * P]
                            nc.tensor.matmul(
                                out=pt,
                                lhsT=w2_sb[0][:, oh * P:(oh + 1) * P],
                                rhs=hts[:, 0:P], start=True, stop=False)
                            nc.tensor.matmul(
                                out=pt,
                                lhsT=w2_sb[1][:, oh * P:(oh + 1) * P],
                                rhs=hts[:, P:D], start=False, stop=True)
                            nc.scalar.activation(
                                out=v2t[:, oh * P:(oh + 1) * P], in_=pt,
                                func=mybir.ActivationFunctionType.Relu,
                                bias=b2_sb[:, oh:oh + 1])
                        pp = wpool.tile([P, F3], f32, tag="psw")
                        nc.tensor.matmul(out=pp[:], lhsT=v2t[:, 0:P],
                                         rhs=w3_sb[0][:], start=True,
                                         stop=False)
                        nc.tensor.matmul(out=pp[:], lhsT=v2t[:, P:D],
                                         rhs=w3_sb[1][:], start=False,
                                         stop=True)
                        xsb = obpool.tile([P, F3], bf16, tag="xw")
                        nc.vector.tensor_copy(xsb[:], pp[:])
                        if debug == "x3":
                            nc.sync.dma_start(out=out_d[P * b:P * b + cnt, :],
                                              in_=xsb[:cnt, :])
                        else:
                            nc.sync.dma_start(
                                out=xw3_d[P * b:P * b + cnt, 0:F3],
                                in_=xsb[:cnt, :])

                gather_chunks(etab, idxv_sb, lucv_sb, tiles_v, TV, D,
                              on_tile)

            # ---------------- node pass, layer 2 (final + log_softmax)
            def node_pass_final(etab):
                psums = {}

                def on_tile(j, g, st, b, first, last):
                    if first:
                        psums[b] = wpool.tile([P, F3], f32, tag="psw",
                                              name=f"pf{b}")
                    nc.tensor.matmul(
                        out=psums[b][:],
                        lhsT=st[:, j * P:(j + 1) * P],
                        rhs=g[:, j * LP:j * LP + F3],
                        start=first, stop=last,
                    )
                    if last:
                        finish(b, psums.pop(b))

                def finish(b, ps):
                    cnt = min(P, VPC - P * b)
                    osb = smpool.tile([P, F3], f32, tag="osb")
                    nc.vector.tensor_scalar_mul(osb[:], ps[:],
                                                rdv_sb[:, b:b + 1])
                    nc.vector.tensor_tensor(out=osb[:], in0=osb[:],
                                            in1=b3_sb[:],
                                            op=mybir.AluOpType.add)
                    negmax = smpool.tile([P, 1], f32, tag="negmax")
                    nc.vector.tensor_reduce(
                        out=negmax[:], in_=osb[:], axis=mybir.AxisListType.X,
                        op=mybir.AluOpType.max, negate=True)
                    expt = smpool.tile([P, F3], f32, tag="expt")
                    sumexp = smpool.tile([P, 1], f32, tag="sumexp")
                    nc.scalar.activation(
                        out=expt[:], in_=osb[:],
                        func=mybir.ActivationFunctionType.Exp,
                        bias=negmax[:, 0:1], accum_out=sumexp[:, 0:1])
                    logsum = smpool.tile([P, 1], f32, tag="logsum")
                    nc.scalar.activation(
                        out=logsum[:], in_=sumexp[:],
                        func=mybir.ActivationFunctionType.Ln)
                    shift = smpool.tile([P, 1], f32, tag="shift")
                    nc.vector.tensor_sub(out=shift[:], in0=negmax[:],
                                         in1=logsum[:])
                    res = smpool.tile([P, F3], f32, tag="res")
                    nc.vector.tensor_scalar_add(res[:], osb[:],
                                                shift[:, 0:1])
                    nc.sync.dma_start(out=out_d[P * b:P * b + cnt, :],
                                      in_=res[:cnt, :])

                gather_chunks(etab, idxv_sb, lucv_sb, tiles_v, TV, LP,
                              on_tile)

            if debug == "g0":
                gf = gpool.tile([P, TC * D // 2], f32, tag="g")
                nc.gpsimd.indirect_dma_start(
                    out=gf[:], out_offset=None,
                    in_=xloc_d[:].bitcast(f32),
                    in_offset=bass.IndirectOffsetOnAxis(
                        ap=idxe_sb[:, 0:TC], axis=0),
                )
                g = gf[:].bitcast(bf16)
                st = spool.tile([P, TC * P], bf16, tag="st")
                nc.vector.tensor_tensor(
                    out=st[:].rearrange("p (t i) -> p t i", i=P),
                    in0=luce_sb[:, 0:TC]
                        .unsqueeze(2).to_broadcast([P, TC, P]),
                    in1=iota_sb[:].unsqueeze(1).to_broadcast([P, TC, P]),
                    op=mybir.AluOpType.is_equal,
                )
                nc.sync.dma_start(out=out_d[:, 0:TC * D], in_=g)
                nc.sync.dma_start(out=out_d[:, TC * D:], in_=st[:])
                layers = []
            else:
                layers = list(range(3))

            # ---------------- the three layers
            for layer in layers:
                table = (xloc_d, vloc1_d, xw3_d)[layer]
                F = D if layer < 2 else F3
                TW = D if layer < 2 else LP
                edge_pass(table, F, TW, part_d[layer],
                          dump=out_d if (debug == "p0" and layer == 0)
                          else None)
                if debug == "p0":
                    break
                nc.gpsimd.collective_compute(
                    "AllReduce", mybir.AluOpType.add, replica_groups=groups,
                    ins=[part_d[layer][:].opt()],
                    outs=[etab_d[layer][:].opt()],
                )
                if debug == "e0":
                    for b in range(NBE):
                        t = eopool.tile([P, D], bf16, tag="eo")
                        nc.sync.dma_start(
                            out=t[:], in_=etab_d[0][P * b:P * (b + 1), :])
                        nc.sync.dma_start(
                            out=out_d[P * b:P * (b + 1), :], in_=t[:])
                    break
                if layer < 2:
                    node_pass_t(etab_d[layer], layer)
                else:
                    node_pass_final(etab_d[layer])
                if debug == "v0" and layer == 0:
                    break
                if debug == "x3" and layer == 1:
                    break
    nc.finalize()
    return nc


def build_in_maps(meta, X, W1, b1, W2, b2, W3, b3):
    import ml_dtypes
    bf16 = ml_dtypes.bfloat16

    X = np.asarray(X, dtype=np.float32)
    iota = np.broadcast_to(np.arange(P, dtype=np.float32),
                           (P, P)).astype(bf16)
    ws = [np.ascontiguousarray(np.asarray(w, np.float32)).astype(bf16)
          for w in (W1, W2, W3)]
    b1x = np.broadcast_to(np.asarray(b1, np.float32), (P, D)).copy()
    b2a = np.asarray(b2, np.float32)
    b2x = np.stack([b2a[0:P], b2a[P:2 * P]], axis=1).copy()  # [128, 2]
    b3x = np.broadcast_to(np.asarray(b3, np.float32), (P, F3)).copy()

    in_maps = []
    for c in range(NCORES):
        pc = meta["per_core"][c]
        xloc = np.ascontiguousarray(X[c * VPC:(c + 1) * VPC]).astype(bf16)
        in_maps.append({
            "xloc": xloc, "idxe": pc["idxe"], "luce": pc["luce"],
            "idxv": pc["idxv"], "lucv": pc["lucv"],
            "rde": meta["rde"], "rdv": pc["rdv"], "rdvt": pc["rdvT"],
            "w1": ws[0], "w2": ws[1], "w3": ws[2],
            "b1x": b1x, "b2x": b2x, "b3x": b3x,
            "iota": iota,
        })
    return in_maps


_CACHE = {}


def kernel(X, node_idx, edge_idx, W1, b1, W2, b2, W3, b3):
    from concourse import bass_utils

    ni = np.asarray(node_idx, dtype=np.int32)
    ei = np.asarray(edge_idx, dtype=np.int32)

    key = hashlib.sha1(ni.tobytes() + ei.tobytes()).hexdigest()
    if key not in _CACHE:
        meta = _preprocess(ni, ei)
        nc = _build(meta)
        _CACHE[key] = (meta, nc)
    meta, nc = _CACHE[key]

    in_maps = build_in_maps(meta, X, W1, b1, W2, b2, W3, b3)
    res = bass_utils.run_bass_kernel_spmd(nc, in_maps, list(range(NCORES)))
    return np.concatenate([res.results[c]["out"] for c in range(NCORES)],
                          axis=0)
